# revision 6
# baseline (speedup 1.0000x reference)
"""BiasedMHA Trainium2 kernel.

Full inputs -> shard batch over 8 NeuronCores -> Bass/Tile kernel -> gather.

Reference semantics (B=16, N=512, F=512, H=16, D=32):
  q = (x @ Wq.T + bq) * sqrt(D); k = x @ Wk.T + bk; v = x @ Wv.T + bv
  s[b,q,k,h] = sum_d q.k + bias[b,q,k,h];  s = -inf where mask[b,q,k]!=0
  p = softmax_k(s);  out = (p @ v reshaped) @ Wo.T + bo

Per-core design notes:
 - X^T via PE transpose; projections as W^T-stationary fp32 matmuls.
 - V kept in natural (n, f) layout, augmented with a ones column per head so
   the P@V matmul also emits the softmax denominator (M=33) for free.
 - Scores stay q-major for the (q,k,h)-contiguous bias tile add + int mask
   predication, then are PE-transposed to k-major so the exp (ScalarE) writes
   P^T straight to SBUF for the P@V matmul - no PSUM->SBUF copy for P.
 - softmax uses a fixed exp shift (exp(s - C)) instead of a row max: scores
   are bounded (std ~16) so exp stays in fp32 range and the shift cancels.
 - Normalization (1/rowsum) is folded into the attn^T PSUM->SBUF copies.
"""

import os
import numpy as np
from contextlib import ExitStack

import concourse.bass as bass
import concourse.mybir as mybir
import concourse.tile as tile
from concourse import bacc
from concourse.bass_utils import run_bass_kernel_spmd
from concourse.masks import make_identity

F32 = mybir.dt.float32
I32 = mybir.dt.int32
ADD = mybir.AluOpType.add
MULT = mybir.AluOpType.mult
AF = mybir.ActivationFunctionType

B, N, F, H = 16, 512, 512, 16
D = F // H            # 32
NCORES = 8
BLOC = B // NCORES    # 2
P = 128
QT = N // P           # 4 q tiles
KC = N // P           # 4 k chunks
SQRT_D = float(np.sqrt(D))
C_EXP = 90.0          # fixed softmax shift; |scores| << C_EXP + 87 (fp32 safe)
NEG_HUGE = -1.0e30


def _emit(nc, tc, ctx, t):
    consts = ctx.enter_context(tc.tile_pool(name="consts", bufs=1))
    wpool = ctx.enter_context(tc.tile_pool(name="weights", bufs=1))
    xpool = ctx.enter_context(tc.tile_pool(name="x", bufs=5))
    bpool = ctx.enter_context(tc.tile_pool(name="perbatch", bufs=1))
    biaspool = ctx.enter_context(tc.tile_pool(name="bias", bufs=2))
    maskpool = ctx.enter_context(tc.tile_pool(name="mask", bufs=2))
    spool = ctx.enter_context(tc.tile_pool(name="sprime", bufs=3))
    ppool = ctx.enter_context(tc.tile_pool(name="pT", bufs=3))
    atsb = ctx.enter_context(tc.tile_pool(name="attnT", bufs=2))
    opool = ctx.enter_context(tc.tile_pool(name="o", bufs=2))
    rspool = ctx.enter_context(tc.tile_pool(name="rs", bufs=2))

    ps_sc = ctx.enter_context(tc.tile_pool(name="ps_sc", bufs=2, space="PSUM"))
    ps_sT = ctx.enter_context(tc.tile_pool(name="ps_sT", bufs=2, space="PSUM"))
    ps_at = ctx.enter_context(tc.tile_pool(name="ps_at", bufs=2, space="PSUM"))
    ps_mi = ctx.enter_context(tc.tile_pool(name="ps_mi", bufs=2, space="PSUM"))

    ident = consts.tile([P, P], F32)
    make_identity(nc, ident[:])
    neghuge = consts.tile([P, 1], F32)
    nc.vector.memset(neghuge[:], NEG_HUGE)
    ones_col = consts.tile([1, P], F32)
    nc.vector.memset(ones_col[:], 1.0)
    negc = consts.tile([P, 1], F32)
    nc.vector.memset(negc[:], -C_EXP)

    # per-partition bias vectors for Q/K projection epilogues
    bqs_sb = consts.tile([P, 4], F32)
    nc.sync.dma_start(bqs_sb[:], t["bqs"].rearrange("(a p) -> p a", p=P))
    bk_sb = consts.tile([P, 4], F32)
    nc.sync.dma_start(bk_sb[:], t["bk"].rearrange("(a p) -> p a", p=P))
    bv_row = consts.tile([1, F], F32)
    nc.sync.dma_start(bv_row[:], t["bv"].rearrange("(a f) -> a f", a=1))
    bo_row = consts.tile([1, F], F32)
    nc.sync.dma_start(bo_row[:], t["bo"].rearrange("(a f) -> a f", a=1))

    w_sb = {}
    for name in ("wqT", "wkT", "wvT", "woT"):
        w_sb[name] = []
        for ki in range(4):
            wt = wpool.tile([P, F], F32, tag=f"{name}{ki}")
            nc.sync.dma_start(wt[:], t[name][P * ki : P * (ki + 1), :])
            w_sb[name].append(wt)

    for b in range(BLOC):
        # ---- X load + transpose to (f_in, n)
        x_tiles = []
        for nb in range(4):
            xt_ = xpool.tile([P, F], F32, tag="x")
            nc.sync.dma_start(xt_[:], t["nfeat"][b, P * nb : P * (nb + 1), :])
            x_tiles.append(xt_)
        xT_sb = bpool.tile([P, 4, N], F32, tag="xT")
        for fb in range(4):
            ps = ps_mi.tile([P, N], F32, tag="mi")
            for nb in range(4):
                nc.tensor.transpose(
                    ps[:, P * nb : P * (nb + 1)],
                    x_tiles[nb][:, P * fb : P * (fb + 1)],
                    ident[:],
                )
            nc.scalar.copy(xT_sb[:, fb, :], ps[:])

        # ---- Q/K projections -> (f_out, n), V -> natural (n, f) augmented
        qT_sb = bpool.tile([P, 4, N], F32, tag="qT")
        kT_sb = bpool.tile([P, 4, N], F32, tag="kT")
        for wname, dest, scale, bvec in (
            ("wqT", qT_sb, SQRT_D, bqs_sb),
            ("wkT", kT_sb, 1.0, bk_sb),
        ):
            for fo in range(4):
                ps = ps_mi.tile([P, N], F32, tag="mi")
                for ki in range(4):
                    nc.tensor.matmul(
                        ps[:],
                        w_sb[wname][ki][:, P * fo : P * (fo + 1)],
                        xT_sb[:, ki, :],
                        start=(ki == 0),
                        stop=(ki == 3),
                    )
                nc.scalar.activation(
                    dest[:, fo, :], ps[:], AF.Identity,
                    bias=bvec[:, fo : fo + 1], scale=scale,
                )

        v_aug = bpool.tile([P, 4, H, 2 * D], F32, tag="vaug")
        for nb in range(4):
            ps = ps_mi.tile([P, N], F32, tag="mi")
            for ki in range(4):
                nc.tensor.matmul(
                    ps[:],
                    xT_sb[:, ki, P * nb : P * (nb + 1)],
                    w_sb["wvT"][ki][:],
                    start=(ki == 0),
                    stop=False,
                )
            nc.tensor.matmul(ps[:], ones_col[:], bv_row[:], start=False, stop=True)
            nc.scalar.copy(
                v_aug[:, nb, :, 0:D], ps[:].rearrange("p (h d) -> p h d", h=H)
            )
            nc.vector.memset(v_aug[:, nb, :, D : 2 * D], 1.0)

        # ---- attention per q-tile
        for qt in range(QT):
            bias_t = biaspool.tile([P, N, H], F32, tag="bias")
            nc.sync.dma_start(bias_t[:], t["attn_bias"][b, P * qt : P * (qt + 1), :, :])
            mask_t = maskpool.tile([P, N], I32, tag="mask")
            nc.sync.dma_start(mask_t[:], t["attn_mask"][b, P * qt : P * (qt + 1), :])

            at_ps = [ps_at.tile([P, 4, P], F32, tag="at", name=f"at{qt}_{i}") for i in range(2)]

            for h in range(H):
                tt_ = h // 8
                s_ = (h % 8) // 4
                j_ = h % 4
                fo, ro = h // 4, D * (h % 4)

                sc = ps_sc.tile([P, N], F32, tag="sc")
                kwargs = {}
                if ro == 96:
                    kwargs["tile_position"] = (ro, 0)
                nc.tensor.matmul(
                    sc[:],
                    qT_sb[ro : ro + D, fo, P * qt : P * (qt + 1)],
                    kT_sb[ro : ro + D, fo, :],
                    start=True,
                    stop=True,
                    **kwargs,
                )
                nc.vector.copy_predicated(
                    sc[:], mask_t[:], neghuge[:].to_broadcast([P, N])
                )
                sp = spool.tile([P, N], F32, tag="sp")
                nc.vector.tensor_tensor(sp[:], sc[:], bias_t[:, :, h], op=ADD)

                sT = ps_sT.tile([P, 4, P], F32, tag="sT")
                for c in range(4):
                    nc.tensor.transpose(
                        sT[:, c, :], sp[:, P * c : P * (c + 1)], ident[:]
                    )
                pT = ppool.tile([P, 4, P], F32, tag="pT")
                nc.scalar.activation(pT[:], sT[:], AF.Exp, bias=negc[:], scale=1.0)

                for kc in range(4):
                    nc.tensor.matmul(
                        at_ps[tt_][64 * s_ : 64 * s_ + 2 * D, j_, :],
                        v_aug[:, kc, h, :],
                        pT[:, kc, :],
                        start=(kc == 0),
                        stop=(kc == 3),
                        tile_position=(0, 64 * s_),
                    )

            # ---- replicated rowsums -> reciprocal, then normalized attn^T
            rc = [rspool.tile([2 * D, 4, P], F32, tag=f"rc{i}", name=f"rc{qt}_{i}") for i in range(2)]
            for tt_ in range(2):
                for s_ in range(2):
                    nc.vector.reciprocal(
                        rc[tt_][D * s_ : D * (s_ + 1), :, :],
                        at_ps[tt_][64 * s_ + D : 64 * s_ + 2 * D, :, :],
                    )

            attnT_sb = atsb.tile([P, 4, P], F32, tag="attnT")
            for h in range(H):
                tt_ = h // 8
                s_ = (h % 8) // 4
                j_ = h % 4
                g, ro = h // 4, D * (h % 4)
                nc.vector.tensor_tensor(
                    attnT_sb[ro : ro + D, g, :],
                    at_ps[tt_][64 * s_ : 64 * s_ + D, j_, :],
                    rc[tt_][D * s_ : D * (s_ + 1), j_, :],
                    op=MULT,
                )

            # ---- output projection
            ps_o = ps_mi.tile([P, N], F32, tag="mi")
            for g in range(4):
                nc.tensor.matmul(
                    ps_o[:],
                    attnT_sb[:, g, :],
                    w_sb["woT"][g][:],
                    start=(g == 0),
                    stop=False,
                )
            nc.tensor.matmul(ps_o[:], ones_col[:], bo_row[:], start=False, stop=True)
            o_sb = opool.tile([P, N], F32, tag="o")
            nc.scalar.copy(o_sb[:], ps_o[:])
            nc.sync.dma_start(t["out"][b, P * qt : P * (qt + 1), :], o_sb[:])


_PROG = None


def _get_prog():
    global _PROG
    if _PROG is None:
        nc = bacc.Bacc("TRN2", target_bir_lowering=False, debug=False,
                       num_devices=NCORES)
        t = {
            "nfeat": nc.dram_tensor("nfeat", [BLOC, N, F], F32, kind="ExternalInput").ap(),
            "attn_bias": nc.dram_tensor("attn_bias", [BLOC, N, N, H], F32, kind="ExternalInput").ap(),
            "attn_mask": nc.dram_tensor("attn_mask", [BLOC, N, N], I32, kind="ExternalInput").ap(),
            "wqT": nc.dram_tensor("wqT", [F, F], F32, kind="ExternalInput").ap(),
            "wkT": nc.dram_tensor("wkT", [F, F], F32, kind="ExternalInput").ap(),
            "wvT": nc.dram_tensor("wvT", [F, F], F32, kind="ExternalInput").ap(),
            "woT": nc.dram_tensor("woT", [F, F], F32, kind="ExternalInput").ap(),
            "bqs": nc.dram_tensor("bqs", [F], F32, kind="ExternalInput").ap(),
            "bk": nc.dram_tensor("bk", [F], F32, kind="ExternalInput").ap(),
            "bv": nc.dram_tensor("bv", [F], F32, kind="ExternalInput").ap(),
            "bo": nc.dram_tensor("bo", [F], F32, kind="ExternalInput").ap(),
            "out": nc.dram_tensor("out", [BLOC, N, F], F32, kind="ExternalOutput").ap(),
        }
        with tile.TileContext(nc) as tc, ExitStack() as ctx:
            _emit(nc, tc, ctx, t)
        nc.compile()
        _PROG = nc
    return _PROG


def kernel(nfeat, attn_bias, attn_mask, Wq, bq, Wk, bk, Wv, bv, Wo, bo):
    nc = _get_prog()
    nfeat = np.ascontiguousarray(np.asarray(nfeat, dtype=np.float32))
    attn_bias = np.ascontiguousarray(np.asarray(attn_bias, dtype=np.float32))
    attn_mask = np.ascontiguousarray(np.asarray(attn_mask, dtype=np.int32))
    shared = {
        "wqT": np.ascontiguousarray(np.asarray(Wq, dtype=np.float32).T),
        "wkT": np.ascontiguousarray(np.asarray(Wk, dtype=np.float32).T),
        "wvT": np.ascontiguousarray(np.asarray(Wv, dtype=np.float32).T),
        "woT": np.ascontiguousarray(np.asarray(Wo, dtype=np.float32).T),
        "bqs": np.asarray(bq, dtype=np.float32) * SQRT_D,
        "bk": np.asarray(bk, dtype=np.float32),
        "bv": np.asarray(bv, dtype=np.float32),
        "bo": np.asarray(bo, dtype=np.float32),
    }
    in_maps = []
    for c in range(NCORES):
        m = dict(shared)
        m["nfeat"] = nfeat[BLOC * c : BLOC * (c + 1)]
        m["attn_bias"] = attn_bias[BLOC * c : BLOC * (c + 1)]
        m["attn_mask"] = attn_mask[BLOC * c : BLOC * (c + 1)]
        in_maps.append(m)

    kernel.last_in_maps = in_maps
    trace = bool(int(os.environ.get("KERNEL_TRACE", "0")))
    res = run_bass_kernel_spmd(
        nc, in_maps, core_ids=list(range(NCORES)), trace=trace
    )
    if trace:
        kernel.last_exec_time_ns = res.exec_time_ns
        kernel.last_profile = res.profile_json
    out = np.concatenate([r["out"] for r in res.results], axis=0)
    return out.astype(np.float32)


kernel.last_exec_time_ns = None
kernel.last_profile = None
kernel.last_in_maps = None


# revision 9
# speedup vs baseline: 1.2429x; 1.2429x over previous
"""BiasedMHA Trainium2 kernel.

Full inputs -> shard batch over 8 NeuronCores -> Bass/Tile kernel -> gather.

Reference semantics (B=16, N=512, F=512, H=16, D=32):
  q = (x @ Wq.T + bq) * sqrt(D); k = x @ Wk.T + bk; v = x @ Wv.T + bv
  s[b,q,k,h] = sum_d q.k + bias[b,q,k,h];  s = -inf where mask[b,q,k]!=0
  p = softmax_k(s);  out = (p @ v reshaped) @ Wo.T + bo

Per-core design notes:
 - X^T via PE transpose; projections as W^T-stationary fp32 matmuls.
 - V kept in natural (n, f) layout, augmented with a ones column per head so
   the P@V matmul also emits the softmax denominator (M=33) for free.
 - Scores stay q-major for the (q,k,h)-contiguous bias tile add + int mask
   predication, then are PE-transposed to k-major so the exp (ScalarE) writes
   P^T straight to SBUF for the P@V matmul - no PSUM->SBUF copy for P.
 - softmax uses a fixed exp shift (exp(s - C)) instead of a row max: scores
   are bounded (std ~16) so exp stays in fp32 range and the shift cancels.
 - Normalization (1/rowsum) is folded into the attn^T PSUM->SBUF copies.
"""

import os
import numpy as np
from contextlib import ExitStack

import concourse.bass as bass
import concourse.mybir as mybir
import concourse.tile as tile
from concourse import bacc
from concourse.bass_utils import run_bass_kernel_spmd
from concourse.masks import make_identity

F32 = mybir.dt.float32
F32R = mybir.dt.float32r
I32 = mybir.dt.int32
ADD = mybir.AluOpType.add
MULT = mybir.AluOpType.mult
AF = mybir.ActivationFunctionType

B, N, F, H = 16, 512, 512, 16
D = F // H            # 32
NCORES = 8
BLOC = B // NCORES    # 2
P = 128
QT = N // P           # 4 q tiles
KC = N // P           # 4 k chunks
SQRT_D = float(np.sqrt(D))
C_EXP = 90.0          # fixed softmax shift; |scores| << C_EXP + 87 (fp32 safe)
NEG_HUGE = -1.0e30


def _emit(nc, tc, ctx, t, reps=1):
    consts = ctx.enter_context(tc.tile_pool(name="consts", bufs=1))
    wpool = ctx.enter_context(tc.tile_pool(name="weights", bufs=1))
    xpool = ctx.enter_context(tc.tile_pool(name="x", bufs=5))
    bpool = ctx.enter_context(tc.tile_pool(name="perbatch", bufs=1))
    biaspool = ctx.enter_context(tc.tile_pool(name="bias", bufs=2))
    maskpool = ctx.enter_context(tc.tile_pool(name="mask", bufs=2))
    mcpool = ctx.enter_context(tc.tile_pool(name="maskC", bufs=2))
    spool = ctx.enter_context(tc.tile_pool(name="sprime", bufs=3))
    ppool = ctx.enter_context(tc.tile_pool(name="pT", bufs=3))
    atsb = ctx.enter_context(tc.tile_pool(name="attnT", bufs=2))
    opool = ctx.enter_context(tc.tile_pool(name="o", bufs=2))
    rspool = ctx.enter_context(tc.tile_pool(name="rs", bufs=2))

    ps_sc = ctx.enter_context(tc.tile_pool(name="ps_sc", bufs=4, space="PSUM"))
    ps_at = ctx.enter_context(tc.tile_pool(name="ps_at", bufs=2, space="PSUM"))
    ps_mi = ctx.enter_context(tc.tile_pool(name="ps_mi", bufs=2, space="PSUM"))

    ident = consts.tile([P, P], F32)
    make_identity(nc, ident[:])
    neghuge = consts.tile([P, 1], F32)
    nc.vector.memset(neghuge[:], NEG_HUGE)
    ones_col = consts.tile([1, P], F32)
    nc.vector.memset(ones_col[:], 1.0)
    ones_r = consts.tile([1, P], F32R)
    nc.vector.tensor_copy(ones_r[:], ones_col[:])
    negc = consts.tile([P, 1], F32)
    nc.vector.memset(negc[:], -C_EXP)

    # per-partition bias vectors for Q/K projection epilogues
    bqs_sb = consts.tile([P, 4], F32)
    nc.sync.dma_start(bqs_sb[:], t["bqs"].rearrange("(a p) -> p a", p=P))
    bk_sb = consts.tile([P, 4], F32)
    nc.sync.dma_start(bk_sb[:], t["bk"].rearrange("(a p) -> p a", p=P))
    bv_row0 = consts.tile([1, F], F32)
    nc.sync.dma_start(bv_row0[:], t["bv"].rearrange("(a f) -> a f", a=1))
    bv_row = consts.tile([1, F], F32R)
    nc.vector.tensor_copy(bv_row[:], bv_row0[:])
    bo_row0 = consts.tile([1, F], F32)
    nc.sync.dma_start(bo_row0[:], t["bo"].rearrange("(a f) -> a f", a=1))
    bo_row = consts.tile([1, F], F32R)
    nc.vector.tensor_copy(bo_row[:], bo_row0[:])

    w_sb = {}
    for name in ("wqT", "wkT", "wvT", "woT"):
        w_sb[name] = []
        for ki in range(4):
            wt = wpool.tile([P, F], F32, tag=f"{name}{ki}")
            nc.sync.dma_start(wt[:], t[name][P * ki : P * (ki + 1), :])
            w_sb[name].append(wt)

    for rep in range(reps):
      for b in range(BLOC):
        # ---- X load + transpose to (f_in, n)
        x_tiles = []
        for nb in range(4):
            xt_ = xpool.tile([P, F], F32, tag="x")
            nc.sync.dma_start(xt_[:], t["nfeat"][b, P * nb : P * (nb + 1), :])
            x_tiles.append(xt_)
        xT_sb = bpool.tile([P, 4, N], F32, tag="xT")
        for fb in range(4):
            ps = ps_mi.tile([P, N], F32, tag="mi")
            for nb in range(4):
                nc.tensor.transpose(
                    ps[:, P * nb : P * (nb + 1)],
                    x_tiles[nb][:, P * fb : P * (fb + 1)],
                    ident[:],
                )
            nc.scalar.copy(xT_sb[:, fb, :], ps[:])

        # ---- Q/K projections -> (f_out, n), V -> natural (n, f) augmented
        qT_sb = bpool.tile([P, 4, N], F32, tag="qT")
        kT_sb = bpool.tile([P, 4, N], F32, tag="kT")
        for wname, dest, scale, bvec in (
            ("wqT", qT_sb, SQRT_D, bqs_sb),
            ("wkT", kT_sb, 1.0, bk_sb),
        ):
            for fo in range(4):
                ps = ps_mi.tile([P, N], F32, tag="mi")
                for ki in range(4):
                    nc.tensor.matmul(
                        ps[:],
                        w_sb[wname][ki][:, P * fo : P * (fo + 1)],
                        xT_sb[:, ki, :],
                        start=(ki == 0),
                        stop=(ki == 3),
                    )
                nc.scalar.activation(
                    dest[:, fo, :], ps[:], AF.Identity,
                    bias=bvec[:, fo : fo + 1], scale=scale,
                )

        v_aug = bpool.tile([P, 4, H, 2 * D], F32, tag="vaug")
        for nb in range(4):
            ps = ps_mi.tile([P, N], F32, tag="mi")
            for ki in range(4):
                nc.tensor.matmul(
                    ps[:],
                    xT_sb[:, ki, P * nb : P * (nb + 1)],
                    w_sb["wvT"][ki][:],
                    start=(ki == 0),
                    stop=False,
                )
            nc.tensor.matmul(ps[:], ones_r[:], bv_row[:], start=False, stop=True)
            nc.scalar.copy(
                v_aug[:, nb, :, 0:D], ps[:].rearrange("p (h d) -> p h d", h=H)
            )
            nc.vector.memset(v_aug[:, nb, :, D : 2 * D], 1.0)

        # ---- attention per q-tile
        for qt in range(QT):
            bias_t = biaspool.tile([P, N, H], F32, tag="bias")
            nc.sync.dma_start(bias_t[:], t["attn_bias"][b, P * qt : P * (qt + 1), :, :])
            mask_t = maskpool.tile([P, N], I32, tag="mask")
            nc.sync.dma_start(mask_t[:], t["attn_mask"][b, P * qt : P * (qt + 1), :])
            maskf = mcpool.tile([P, N], F32, tag="maskf")
            nc.vector.tensor_copy(maskf[:], mask_t[:])
            maskC = mcpool.tile([P, N, 1], F32, tag="maskC")
            nc.vector.tensor_scalar(
                maskC[:, :, 0], maskf[:], 0.0, NEG_HUGE,
                op0=mybir.AluOpType.not_equal, op1=MULT,
            )
            nc.vector.tensor_tensor(
                bias_t[:], bias_t[:], maskC[:].to_broadcast([P, N, H]), op=ADD
            )

            at_ps = [ps_at.tile([P, 4, P], F32, tag="at", name=f"at{qt}_{i}") for i in range(2)]

            for h in range(H):
                tt_ = h // 8
                s_ = h % 2
                j_ = (h % 8) // 2
                fo, ro = h // 4, D * (h % 4)

                sc = ps_sc.tile([P, N], F32, tag="sc")
                kwargs = {}
                if ro == 96:
                    kwargs["tile_position"] = (ro, 0)
                nc.tensor.matmul(
                    sc[:],
                    qT_sb[ro : ro + D, fo, P * qt : P * (qt + 1)],
                    kT_sb[ro : ro + D, fo, :],
                    start=True,
                    stop=True,
                    **kwargs,
                )
                sp = spool.tile([P, N], F32, tag="sp")
                nc.vector.tensor_tensor(sp[:], sc[:], bias_t[:, :, h], op=ADD)

                for c in range(4):
                    nc.tensor.transpose(
                        sc[:, P * c : P * (c + 1)], sp[:, P * c : P * (c + 1)], ident[:]
                    )
                pT = ppool.tile([P, 4, P], F32, tag="pT")
                nc.scalar.activation(
                    pT[:], sc[:].rearrange("p (c q) -> p c q", c=4),
                    AF.Exp, bias=negc[:], scale=1.0,
                )

                for kc in range(4):
                    nc.tensor.matmul(
                        at_ps[tt_][64 * s_ : 64 * s_ + 2 * D, j_, :],
                        v_aug[:, kc, h, :],
                        pT[:, kc, :],
                        start=(kc == 0),
                        stop=(kc == 3),
                        tile_position=(0, 64 * s_),
                    )

            # ---- replicated rowsums -> reciprocal, then normalized attn^T
            rc = [rspool.tile([2 * D, 4, P], F32, tag=f"rc{i}", name=f"rc{qt}_{i}") for i in range(2)]
            for tt_ in range(2):
                for s_ in range(2):
                    nc.vector.reciprocal(
                        rc[tt_][D * s_ : D * (s_ + 1), :, :],
                        at_ps[tt_][64 * s_ + D : 64 * s_ + 2 * D, :, :],
                    )

            attnT_sb = atsb.tile([P, 4, P], F32, tag="attnT")
            for h in range(H):
                tt_ = h // 8
                s_ = h % 2
                j_ = (h % 8) // 2
                g, ro = h // 4, D * (h % 4)
                nc.vector.tensor_tensor(
                    attnT_sb[ro : ro + D, g, :],
                    at_ps[tt_][64 * s_ : 64 * s_ + D, j_, :],
                    rc[tt_][D * s_ : D * (s_ + 1), j_, :],
                    op=MULT,
                )

            # ---- output projection
            ps_o = ps_mi.tile([P, N], F32, tag="mi")
            for g in range(4):
                nc.tensor.matmul(
                    ps_o[:],
                    attnT_sb[:, g, :],
                    w_sb["woT"][g][:],
                    start=(g == 0),
                    stop=False,
                )
            nc.tensor.matmul(ps_o[:], ones_r[:], bo_row[:], start=False, stop=True)
            o_sb = opool.tile([P, N], F32, tag="o")
            nc.scalar.copy(o_sb[:], ps_o[:])
            nc.sync.dma_start(t["out"][b, P * qt : P * (qt + 1), :], o_sb[:])


_PROG = None


def _get_prog(reps=1):
    global _PROG
    if reps != 1:
        return _build_prog(reps)
    if _PROG is None:
        _PROG = _build_prog(1)
    return _PROG


def _build_prog(reps):
        nc = bacc.Bacc("TRN2", target_bir_lowering=False, debug=False,
                       num_devices=NCORES)
        t = {
            "nfeat": nc.dram_tensor("nfeat", [BLOC, N, F], F32, kind="ExternalInput").ap(),
            "attn_bias": nc.dram_tensor("attn_bias", [BLOC, N, N, H], F32, kind="ExternalInput").ap(),
            "attn_mask": nc.dram_tensor("attn_mask", [BLOC, N, N], I32, kind="ExternalInput").ap(),
            "wqT": nc.dram_tensor("wqT", [F, F], F32, kind="ExternalInput").ap(),
            "wkT": nc.dram_tensor("wkT", [F, F], F32, kind="ExternalInput").ap(),
            "wvT": nc.dram_tensor("wvT", [F, F], F32, kind="ExternalInput").ap(),
            "woT": nc.dram_tensor("woT", [F, F], F32, kind="ExternalInput").ap(),
            "bqs": nc.dram_tensor("bqs", [F], F32, kind="ExternalInput").ap(),
            "bk": nc.dram_tensor("bk", [F], F32, kind="ExternalInput").ap(),
            "bv": nc.dram_tensor("bv", [F], F32, kind="ExternalInput").ap(),
            "bo": nc.dram_tensor("bo", [F], F32, kind="ExternalInput").ap(),
            "out": nc.dram_tensor("out", [BLOC, N, F], F32, kind="ExternalOutput").ap(),
        }
        with tile.TileContext(nc) as tc, ExitStack() as ctx:
            _emit(nc, tc, ctx, t, reps=reps)
        nc.compile()
        return nc


def kernel(nfeat, attn_bias, attn_mask, Wq, bq, Wk, bk, Wv, bv, Wo, bo):
    nc = _get_prog()
    nfeat = np.ascontiguousarray(np.asarray(nfeat, dtype=np.float32))
    attn_bias = np.ascontiguousarray(np.asarray(attn_bias, dtype=np.float32))
    attn_mask = np.ascontiguousarray(np.asarray(attn_mask, dtype=np.int32))
    shared = {
        "wqT": np.ascontiguousarray(np.asarray(Wq, dtype=np.float32).T),
        "wkT": np.ascontiguousarray(np.asarray(Wk, dtype=np.float32).T),
        "wvT": np.ascontiguousarray(np.asarray(Wv, dtype=np.float32).T),
        "woT": np.ascontiguousarray(np.asarray(Wo, dtype=np.float32).T),
        "bqs": np.asarray(bq, dtype=np.float32) * SQRT_D,
        "bk": np.asarray(bk, dtype=np.float32),
        "bv": np.asarray(bv, dtype=np.float32),
        "bo": np.asarray(bo, dtype=np.float32),
    }
    in_maps = []
    for c in range(NCORES):
        m = dict(shared)
        m["nfeat"] = nfeat[BLOC * c : BLOC * (c + 1)]
        m["attn_bias"] = attn_bias[BLOC * c : BLOC * (c + 1)]
        m["attn_mask"] = attn_mask[BLOC * c : BLOC * (c + 1)]
        in_maps.append(m)

    kernel.last_in_maps = in_maps
    trace = bool(int(os.environ.get("KERNEL_TRACE", "0")))
    res = run_bass_kernel_spmd(
        nc, in_maps, core_ids=list(range(NCORES)), trace=trace
    )
    if trace:
        kernel.last_exec_time_ns = res.exec_time_ns
        kernel.last_profile = res.profile_json
    out = np.concatenate([r["out"] for r in res.results], axis=0)
    return out.astype(np.float32)


kernel.last_exec_time_ns = None
kernel.last_profile = None
kernel.last_in_maps = None


# revision 11
# speedup vs baseline: 1.5760x; 1.2680x over previous
"""BiasedMHA Trainium2 kernel.

Full inputs -> shard batch over 8 NeuronCores -> Bass/Tile kernel -> gather.

Reference semantics (B=16, N=512, F=512, H=16, D=32):
  q = (x @ Wq.T + bq) * sqrt(D); k = x @ Wk.T + bk; v = x @ Wv.T + bv
  s[b,q,k,h] = sum_d q.k + bias[b,q,k,h];  s = -inf where mask[b,q,k]!=0
  p = softmax_k(s);  out = (p @ v reshaped) @ Wo.T + bo

Per-core design notes:
 - X^T via PE transpose; projections as W^T-stationary fp32 matmuls.
 - V kept in natural (n, f) layout, augmented with a ones column per head so
   the P@V matmul also emits the softmax denominator (M=33) for free.
 - Scores stay q-major for the (q,k,h)-contiguous bias tile add + int mask
   predication, then are PE-transposed to k-major so the exp (ScalarE) writes
   P^T straight to SBUF for the P@V matmul - no PSUM->SBUF copy for P.
 - softmax uses a fixed exp shift (exp(s - C)) instead of a row max: scores
   are bounded (std ~16) so exp stays in fp32 range and the shift cancels.
 - Normalization (1/rowsum) is folded into the attn^T PSUM->SBUF copies.
"""

import os
import numpy as np
from contextlib import ExitStack

import concourse.bass as bass
import concourse.mybir as mybir
import concourse.tile as tile
from concourse import bacc
from concourse.bass_utils import run_bass_kernel_spmd
from concourse.masks import make_identity

F32 = mybir.dt.float32
F32R = mybir.dt.float32r
I32 = mybir.dt.int32
ADD = mybir.AluOpType.add
MULT = mybir.AluOpType.mult
AF = mybir.ActivationFunctionType

B, N, F, H = 16, 512, 512, 16
D = F // H            # 32
NCORES = 8
BLOC = B // NCORES    # 2
P = 128
QT = N // P           # 4 q tiles
KC = N // P           # 4 k chunks
SQRT_D = float(np.sqrt(D))
C_EXP = 90.0          # fixed softmax shift; |scores| << C_EXP + 87 (fp32 safe)
NEG_HUGE = -1.0e30


def _emit(nc, tc, ctx, t, reps=1):
    consts = ctx.enter_context(tc.tile_pool(name="consts", bufs=1))
    wpool = ctx.enter_context(tc.tile_pool(name="weights", bufs=1))
    xpool = ctx.enter_context(tc.tile_pool(name="x", bufs=5))
    bpool = ctx.enter_context(tc.tile_pool(name="perbatch", bufs=1))
    biaspool = ctx.enter_context(tc.tile_pool(name="bias", bufs=2))
    maskpool = ctx.enter_context(tc.tile_pool(name="mask", bufs=2))
    mcpool = ctx.enter_context(tc.tile_pool(name="maskC", bufs=2))
    spool = ctx.enter_context(tc.tile_pool(name="sprime", bufs=4))
    ppool = ctx.enter_context(tc.tile_pool(name="pT", bufs=4))
    atsb = ctx.enter_context(tc.tile_pool(name="attnT", bufs=2))
    opool = ctx.enter_context(tc.tile_pool(name="o", bufs=2))
    rspool = ctx.enter_context(tc.tile_pool(name="rs", bufs=2))

    ps_sc = ctx.enter_context(tc.tile_pool(name="ps_sc", bufs=4, space="PSUM"))
    ps_at = ctx.enter_context(tc.tile_pool(name="ps_at", bufs=2, space="PSUM"))
    ps_mi = ctx.enter_context(tc.tile_pool(name="ps_mi", bufs=2, space="PSUM"))

    ident = consts.tile([P, P], F32)
    make_identity(nc, ident[:])
    neghuge = consts.tile([P, 1], F32)
    nc.vector.memset(neghuge[:], NEG_HUGE)
    ones_col = consts.tile([1, P], F32)
    nc.vector.memset(ones_col[:], 1.0)
    ones_r = consts.tile([1, P], F32R)
    nc.vector.tensor_copy(ones_r[:], ones_col[:])
    negc = consts.tile([P, 1], F32)
    nc.vector.memset(negc[:], -C_EXP)

    # per-partition bias vectors for Q/K projection epilogues
    bqs_sb = consts.tile([P, 4], F32)
    nc.sync.dma_start(bqs_sb[:], t["bqs"].rearrange("(a p) -> p a", p=P))
    bk_sb = consts.tile([P, 4], F32)
    nc.sync.dma_start(bk_sb[:], t["bk"].rearrange("(a p) -> p a", p=P))
    bv_row0 = consts.tile([1, F], F32)
    nc.sync.dma_start(bv_row0[:], t["bv"].rearrange("(a f) -> a f", a=1))
    bv_row = consts.tile([1, F], F32R)
    nc.vector.tensor_copy(bv_row[:], bv_row0[:])
    bo_row0 = consts.tile([1, F], F32)
    nc.sync.dma_start(bo_row0[:], t["bo"].rearrange("(a f) -> a f", a=1))
    bo_row = consts.tile([1, F], F32R)
    nc.vector.tensor_copy(bo_row[:], bo_row0[:])

    w_sb = {}
    for name in ("wqT", "wkT", "wvT", "woT"):
        w_sb[name] = []
        for ki in range(4):
            wt = wpool.tile([P, F], F32, tag=f"{name}{ki}")
            nc.sync.dma_start(wt[:], t[name][P * ki : P * (ki + 1), :])
            w_sb[name].append(wt)

    for rep in range(reps):
      for b in range(BLOC):
        # ---- X load + transpose to (f_in, n)
        x_tiles = []
        for nb in range(4):
            xt_ = xpool.tile([P, F], F32, tag="x")
            nc.sync.dma_start(xt_[:], t["nfeat"][b, P * nb : P * (nb + 1), :])
            x_tiles.append(xt_)
        xT_sb = bpool.tile([P, 4, N], F32, tag="xT")
        for fb in range(4):
            ps = ps_mi.tile([P, N], F32, tag="mi")
            for nb in range(4):
                nc.tensor.transpose(
                    ps[:, P * nb : P * (nb + 1)],
                    x_tiles[nb][:, P * fb : P * (fb + 1)],
                    ident[:],
                )
            nc.scalar.copy(xT_sb[:, fb, :], ps[:])

        # ---- Q/K projections -> (f_out, n), V -> natural (n, f) augmented
        qT_sb = bpool.tile([P, 4, N], F32, tag="qT")
        kT_sb = bpool.tile([P, 4, N], F32, tag="kT")
        for wname, dest, scale, bvec in (
            ("wqT", qT_sb, SQRT_D, bqs_sb),
            ("wkT", kT_sb, 1.0, bk_sb),
        ):
            for fo in range(4):
                ps = ps_mi.tile([P, N], F32, tag="mi")
                for ki in range(4):
                    nc.tensor.matmul(
                        ps[:],
                        w_sb[wname][ki][:, P * fo : P * (fo + 1)],
                        xT_sb[:, ki, :],
                        start=(ki == 0),
                        stop=(ki == 3),
                    )
                nc.scalar.activation(
                    dest[:, fo, :], ps[:], AF.Identity,
                    bias=bvec[:, fo : fo + 1], scale=scale,
                )

        v_aug = bpool.tile([P, 4, H, 2 * D], F32, tag="vaug")
        for nb in range(4):
            ps = ps_mi.tile([P, N], F32, tag="mi")
            for ki in range(4):
                nc.tensor.matmul(
                    ps[:],
                    xT_sb[:, ki, P * nb : P * (nb + 1)],
                    w_sb["wvT"][ki][:],
                    start=(ki == 0),
                    stop=False,
                )
            nc.tensor.matmul(ps[:], ones_r[:], bv_row[:], start=False, stop=True)
            nc.scalar.copy(
                v_aug[:, nb, :, 0:D], ps[:].rearrange("p (h d) -> p h d", h=H)
            )
            nc.vector.memset(v_aug[:, nb, :, D : 2 * D], 1.0)

        # ---- attention per q-tile
        for qt in range(QT):
            bias_t = biaspool.tile([P, N, H], F32, tag="bias")
            nc.sync.dma_start(bias_t[:], t["attn_bias"][b, P * qt : P * (qt + 1), :, :])
            mask_t = maskpool.tile([P, N], I32, tag="mask")
            nc.sync.dma_start(mask_t[:], t["attn_mask"][b, P * qt : P * (qt + 1), :])
            maskf = mcpool.tile([P, N], F32, tag="maskf")
            nc.gpsimd.tensor_copy(maskf[:], mask_t[:])
            maskC = mcpool.tile([P, N, 1], F32, tag="maskC")
            nc.vector.tensor_scalar(
                maskC[:, :, 0], maskf[:], 0.0, NEG_HUGE,
                op0=mybir.AluOpType.not_equal, op1=MULT,
            )
            nc.gpsimd.tensor_tensor(
                bias_t[:], bias_t[:], maskC[:].to_broadcast([P, N, H]), op=ADD
            )

            at_ps = [ps_at.tile([P, 4, P], F32, tag="at", name=f"at{qt}_{i}") for i in range(2)]

            for h in range(H):
                tt_ = h // 8
                s_ = h % 2
                j_ = (h % 8) // 2
                fo, ro = h // 4, D * (h % 4)

                sc = ps_sc.tile([P, N], F32, tag="sc")
                kwargs = {}
                if ro == 96:
                    kwargs["tile_position"] = (ro, 0)
                nc.tensor.matmul(
                    sc[:],
                    qT_sb[ro : ro + D, fo, P * qt : P * (qt + 1)],
                    kT_sb[ro : ro + D, fo, :],
                    start=True,
                    stop=True,
                    **kwargs,
                )
                sp = spool.tile([P, N], F32, tag="sp")
                nc.vector.tensor_tensor(sp[:], sc[:], bias_t[:, :, h], op=ADD)

                for c in range(4):
                    nc.tensor.transpose(
                        sc[:, P * c : P * (c + 1)], sp[:, P * c : P * (c + 1)], ident[:]
                    )
                pT = ppool.tile([P, 4, P], F32, tag="pT")
                nc.scalar.activation(
                    pT[:], sc[:].rearrange("p (c q) -> p c q", c=4),
                    AF.Exp, bias=negc[:], scale=1.0,
                )

                for kc in range(4):
                    nc.tensor.matmul(
                        at_ps[tt_][64 * s_ : 64 * s_ + 2 * D, j_, :],
                        v_aug[:, kc, h, :],
                        pT[:, kc, :],
                        start=(kc == 0),
                        stop=(kc == 3),
                        tile_position=(0, 64 * s_),
                    )

            # ---- replicated rowsums -> reciprocal, then normalized attn^T
            rc = [rspool.tile([2 * D, 4, P], F32, tag=f"rc{i}", name=f"rc{qt}_{i}") for i in range(2)]
            for tt_ in range(2):
                for s_ in range(2):
                    nc.vector.reciprocal(
                        rc[tt_][D * s_ : D * (s_ + 1), :, :],
                        at_ps[tt_][64 * s_ + D : 64 * s_ + 2 * D, :, :],
                    )

            attnT_sb = atsb.tile([P, 4, P], F32, tag="attnT")
            for h in range(H):
                tt_ = h // 8
                s_ = h % 2
                j_ = (h % 8) // 2
                g, ro = h // 4, D * (h % 4)
                nc.vector.tensor_tensor(
                    attnT_sb[ro : ro + D, g, :],
                    at_ps[tt_][64 * s_ : 64 * s_ + D, j_, :],
                    rc[tt_][D * s_ : D * (s_ + 1), j_, :],
                    op=MULT,
                )

            # ---- output projection
            ps_o = ps_mi.tile([P, N], F32, tag="mi")
            for g in range(4):
                nc.tensor.matmul(
                    ps_o[:],
                    attnT_sb[:, g, :],
                    w_sb["woT"][g][:],
                    start=(g == 0),
                    stop=False,
                )
            nc.tensor.matmul(ps_o[:], ones_r[:], bo_row[:], start=False, stop=True)
            o_sb = opool.tile([P, N], F32, tag="o")
            nc.scalar.copy(o_sb[:], ps_o[:])
            nc.sync.dma_start(t["out"][b, P * qt : P * (qt + 1), :], o_sb[:])


_PROG = None


def _get_prog(reps=1):
    global _PROG
    if reps != 1:
        return _build_prog(reps)
    if _PROG is None:
        _PROG = _build_prog(1)
    return _PROG


def _build_prog(reps):
        nc = bacc.Bacc("TRN2", target_bir_lowering=False, debug=False,
                       num_devices=NCORES)
        t = {
            "nfeat": nc.dram_tensor("nfeat", [BLOC, N, F], F32, kind="ExternalInput").ap(),
            "attn_bias": nc.dram_tensor("attn_bias", [BLOC, N, N, H], F32, kind="ExternalInput").ap(),
            "attn_mask": nc.dram_tensor("attn_mask", [BLOC, N, N], I32, kind="ExternalInput").ap(),
            "wqT": nc.dram_tensor("wqT", [F, F], F32, kind="ExternalInput").ap(),
            "wkT": nc.dram_tensor("wkT", [F, F], F32, kind="ExternalInput").ap(),
            "wvT": nc.dram_tensor("wvT", [F, F], F32, kind="ExternalInput").ap(),
            "woT": nc.dram_tensor("woT", [F, F], F32, kind="ExternalInput").ap(),
            "bqs": nc.dram_tensor("bqs", [F], F32, kind="ExternalInput").ap(),
            "bk": nc.dram_tensor("bk", [F], F32, kind="ExternalInput").ap(),
            "bv": nc.dram_tensor("bv", [F], F32, kind="ExternalInput").ap(),
            "bo": nc.dram_tensor("bo", [F], F32, kind="ExternalInput").ap(),
            "out": nc.dram_tensor("out", [BLOC, N, F], F32, kind="ExternalOutput").ap(),
        }
        with tile.TileContext(nc) as tc, ExitStack() as ctx:
            _emit(nc, tc, ctx, t, reps=reps)
        nc.compile()
        return nc


def kernel(nfeat, attn_bias, attn_mask, Wq, bq, Wk, bk, Wv, bv, Wo, bo):
    nc = _get_prog()
    nfeat = np.ascontiguousarray(np.asarray(nfeat, dtype=np.float32))
    attn_bias = np.ascontiguousarray(np.asarray(attn_bias, dtype=np.float32))
    attn_mask = np.ascontiguousarray(np.asarray(attn_mask, dtype=np.int32))
    shared = {
        "wqT": np.ascontiguousarray(np.asarray(Wq, dtype=np.float32).T),
        "wkT": np.ascontiguousarray(np.asarray(Wk, dtype=np.float32).T),
        "wvT": np.ascontiguousarray(np.asarray(Wv, dtype=np.float32).T),
        "woT": np.ascontiguousarray(np.asarray(Wo, dtype=np.float32).T),
        "bqs": np.asarray(bq, dtype=np.float32) * SQRT_D,
        "bk": np.asarray(bk, dtype=np.float32),
        "bv": np.asarray(bv, dtype=np.float32),
        "bo": np.asarray(bo, dtype=np.float32),
    }
    in_maps = []
    for c in range(NCORES):
        m = dict(shared)
        m["nfeat"] = nfeat[BLOC * c : BLOC * (c + 1)]
        m["attn_bias"] = attn_bias[BLOC * c : BLOC * (c + 1)]
        m["attn_mask"] = attn_mask[BLOC * c : BLOC * (c + 1)]
        in_maps.append(m)

    kernel.last_in_maps = in_maps
    trace = bool(int(os.environ.get("KERNEL_TRACE", "0")))
    res = run_bass_kernel_spmd(
        nc, in_maps, core_ids=list(range(NCORES)), trace=trace
    )
    if trace:
        kernel.last_exec_time_ns = res.exec_time_ns
        kernel.last_profile = res.profile_json
    out = np.concatenate([r["out"] for r in res.results], axis=0)
    return out.astype(np.float32)


kernel.last_exec_time_ns = None
kernel.last_profile = None
kernel.last_in_maps = None


# revision 12
# speedup vs baseline: 3.5826x; 2.2733x over previous
"""BiasedMHA Trainium2 kernel.

Full inputs -> shard batch over 8 NeuronCores -> Bass/Tile kernel -> gather.

Reference semantics (B=16, N=512, F=512, H=16, D=32):
  q = (x @ Wq.T + bq) * sqrt(D); k = x @ Wk.T + bk; v = x @ Wv.T + bv
  s[b,q,k,h] = sum_d q.k + bias[b,q,k,h];  s = -inf where mask[b,q,k]!=0
  p = softmax_k(s);  out = (p @ v reshaped) @ Wo.T + bo

Per-core design notes:
 - X^T via PE transpose; projections as W^T-stationary fp32 matmuls.
 - V kept in natural (n, f) layout, augmented with a ones column per head so
   the P@V matmul also emits the softmax denominator (M=33) for free.
 - Scores stay q-major for the (q,k,h)-contiguous bias tile add + int mask
   predication, then are PE-transposed to k-major so the exp (ScalarE) writes
   P^T straight to SBUF for the P@V matmul - no PSUM->SBUF copy for P.
 - softmax uses a fixed exp shift (exp(s - C)) instead of a row max: scores
   are bounded (std ~16) so exp stays in fp32 range and the shift cancels.
 - Normalization (1/rowsum) is folded into the attn^T PSUM->SBUF copies.
"""

import os
import numpy as np
from contextlib import ExitStack

import concourse.bass as bass
import concourse.mybir as mybir
import concourse.tile as tile
from concourse import bacc
from concourse.bass_utils import run_bass_kernel_spmd
from concourse.masks import make_identity

F32 = mybir.dt.float32
F32R = mybir.dt.float32r
I32 = mybir.dt.int32
ADD = mybir.AluOpType.add
MULT = mybir.AluOpType.mult
AF = mybir.ActivationFunctionType

B, N, F, H = 16, 512, 512, 16
D = F // H            # 32
NCORES = 8
BLOC = B // NCORES    # 2
P = 128
QT = N // P           # 4 q tiles
KC = N // P           # 4 k chunks
SQRT_D = float(np.sqrt(D))
C_EXP = 90.0          # fixed softmax shift; |scores| << C_EXP + 87 (fp32 safe)
NEG_HUGE = -1.0e30


def _emit(nc, tc, ctx, t, reps=1):
    consts = ctx.enter_context(tc.tile_pool(name="consts", bufs=1))
    wpool = ctx.enter_context(tc.tile_pool(name="weights", bufs=1))
    xpool = ctx.enter_context(tc.tile_pool(name="x", bufs=5))
    bpool = ctx.enter_context(tc.tile_pool(name="perbatch", bufs=1))
    biaspool = ctx.enter_context(tc.tile_pool(name="bias", bufs=2))
    maskpool = ctx.enter_context(tc.tile_pool(name="mask", bufs=2))
    mcpool = ctx.enter_context(tc.tile_pool(name="maskC", bufs=2))
    spool = ctx.enter_context(tc.tile_pool(name="sprime", bufs=4))
    ppool = ctx.enter_context(tc.tile_pool(name="pT", bufs=4))
    atsb = ctx.enter_context(tc.tile_pool(name="attnT", bufs=2))
    opool = ctx.enter_context(tc.tile_pool(name="o", bufs=2))
    rspool = ctx.enter_context(tc.tile_pool(name="rs", bufs=2))

    ps_sc = ctx.enter_context(tc.tile_pool(name="ps_sc", bufs=3, space="PSUM"))
    ps_at = ctx.enter_context(tc.tile_pool(name="ps_at", bufs=3, space="PSUM"))
    ps_mi = ctx.enter_context(tc.tile_pool(name="ps_mi", bufs=2, space="PSUM"))

    ident = consts.tile([P, P], F32)
    make_identity(nc, ident[:])
    neghuge = consts.tile([P, 1], F32)
    nc.vector.memset(neghuge[:], NEG_HUGE)
    ones_col = consts.tile([1, P], F32)
    nc.vector.memset(ones_col[:], 1.0)
    ones_r = consts.tile([1, P], F32R)
    nc.vector.tensor_copy(ones_r[:], ones_col[:])
    negc = consts.tile([P, 1], F32)
    nc.vector.memset(negc[:], -C_EXP)

    # per-partition bias vectors for Q/K projection epilogues
    bqs_sb = consts.tile([P, 4], F32)
    nc.sync.dma_start(bqs_sb[:], t["bqs"].rearrange("(a p) -> p a", p=P))
    bk_sb = consts.tile([P, 4], F32)
    nc.sync.dma_start(bk_sb[:], t["bk"].rearrange("(a p) -> p a", p=P))
    bv_row0 = consts.tile([1, F], F32)
    nc.sync.dma_start(bv_row0[:], t["bv"].rearrange("(a f) -> a f", a=1))
    bv_row = consts.tile([1, F], F32R)
    nc.vector.tensor_copy(bv_row[:], bv_row0[:])
    bo_row0 = consts.tile([1, F], F32)
    nc.sync.dma_start(bo_row0[:], t["bo"].rearrange("(a f) -> a f", a=1))
    bo_row = consts.tile([1, F], F32R)
    nc.vector.tensor_copy(bo_row[:], bo_row0[:])

    w_sb = {}
    for name in ("wqT", "wkT", "wvT", "woT"):
        w_sb[name] = []
        for ki in range(4):
            wt = wpool.tile([P, F], F32, tag=f"{name}{ki}")
            nc.sync.dma_start(wt[:], t[name][P * ki : P * (ki + 1), :])
            w_sb[name].append(wt)

    for rep in range(reps):
      for b in range(BLOC):
        # ---- X load + transpose to (f_in, n)
        x_tiles = []
        for nb in range(4):
            xt_ = xpool.tile([P, F], F32, tag="x")
            nc.sync.dma_start(xt_[:], t["nfeat"][b, P * nb : P * (nb + 1), :])
            x_tiles.append(xt_)
        xT_sb = bpool.tile([P, 4, N], F32, tag="xT")
        for fb in range(4):
            ps = ps_mi.tile([P, N], F32, tag="mi")
            for nb in range(4):
                nc.tensor.transpose(
                    ps[:, P * nb : P * (nb + 1)],
                    x_tiles[nb][:, P * fb : P * (fb + 1)],
                    ident[:],
                )
            nc.scalar.copy(xT_sb[:, fb, :], ps[:])

        # ---- Q/K projections -> (f_out, n), V -> natural (n, f) augmented
        qT_sb = bpool.tile([P, 4, N], F32, tag="qT")
        kT_sb = bpool.tile([P, 4, N], F32, tag="kT")
        for wname, dest, scale, bvec in (
            ("wqT", qT_sb, SQRT_D, bqs_sb),
            ("wkT", kT_sb, 1.0, bk_sb),
        ):
            for fo in range(4):
                ps = ps_mi.tile([P, N], F32, tag="mi")
                for ki in range(4):
                    nc.tensor.matmul(
                        ps[:],
                        w_sb[wname][ki][:, P * fo : P * (fo + 1)],
                        xT_sb[:, ki, :],
                        start=(ki == 0),
                        stop=(ki == 3),
                    )
                nc.scalar.activation(
                    dest[:, fo, :], ps[:], AF.Identity,
                    bias=bvec[:, fo : fo + 1], scale=scale,
                )

        v_aug = bpool.tile([P, 4, H, 2 * D], F32, tag="vaug")
        for nb in range(4):
            ps = ps_mi.tile([P, N], F32, tag="mi")
            for ki in range(4):
                nc.tensor.matmul(
                    ps[:],
                    xT_sb[:, ki, P * nb : P * (nb + 1)],
                    w_sb["wvT"][ki][:],
                    start=(ki == 0),
                    stop=False,
                )
            nc.tensor.matmul(ps[:], ones_r[:], bv_row[:], start=False, stop=True)
            nc.scalar.copy(
                v_aug[:, nb, :, 0:D], ps[:].rearrange("p (h d) -> p h d", h=H)
            )
            nc.vector.memset(v_aug[:, nb, :, D : 2 * D], 1.0)

        # ---- attention per q-tile
        for qt in range(QT):
            bias_t = biaspool.tile([P, N, H], F32, tag="bias")
            nc.sync.dma_start(bias_t[:], t["attn_bias"][b, P * qt : P * (qt + 1), :, :])
            mask_t = maskpool.tile([P, N], I32, tag="mask")
            nc.sync.dma_start(mask_t[:], t["attn_mask"][b, P * qt : P * (qt + 1), :])
            maskf = mcpool.tile([P, N], F32, tag="maskf")
            nc.gpsimd.tensor_copy(maskf[:], mask_t[:])
            maskC = mcpool.tile([P, N, 1], F32, tag="maskC")
            nc.vector.tensor_scalar(
                maskC[:, :, 0], maskf[:], 0.0, NEG_HUGE,
                op0=mybir.AluOpType.not_equal, op1=MULT,
            )
            nc.gpsimd.tensor_tensor(
                bias_t[:], bias_t[:], maskC[:].to_broadcast([P, N, H]), op=ADD
            )

            at_ps = [ps_at.tile([P, 4, P], F32, tag="at", name=f"at{qt}_{i}") for i in range(2)]

            for h in range(H):
                tt_ = h // 8
                s_ = h % 2
                j_ = (h % 8) // 2
                fo, ro = h // 4, D * (h % 4)

                sc = ps_sc.tile([P, N], F32, tag="sc")
                kwargs = {}
                if ro == 96:
                    kwargs["tile_position"] = (ro, 0)
                nc.tensor.matmul(
                    sc[:],
                    qT_sb[ro : ro + D, fo, P * qt : P * (qt + 1)],
                    kT_sb[ro : ro + D, fo, :],
                    start=True,
                    stop=True,
                    **kwargs,
                )
                sp = spool.tile([P, N], F32, tag="sp")
                nc.vector.tensor_tensor(sp[:], sc[:], bias_t[:, :, h], op=ADD)

                for c in range(4):
                    nc.tensor.transpose(
                        sc[:, P * c : P * (c + 1)], sp[:, P * c : P * (c + 1)], ident[:]
                    )
                pT = ppool.tile([P, 4, P], F32, tag="pT")
                nc.scalar.activation(
                    pT[:], sc[:].rearrange("p (c q) -> p c q", c=4),
                    AF.Exp, bias=negc[:], scale=1.0,
                )

                for kc in range(4):
                    nc.tensor.matmul(
                        at_ps[tt_][64 * s_ : 64 * s_ + 2 * D, j_, :],
                        v_aug[:, kc, h, :],
                        pT[:, kc, :],
                        start=(kc == 0),
                        stop=(kc == 3),
                        tile_position=(0, 64 * s_),
                    )

            # ---- replicated rowsums -> reciprocal, then normalized attn^T
            rc = [rspool.tile([2 * D, 4, P], F32, tag=f"rc{i}", name=f"rc{qt}_{i}") for i in range(2)]
            for tt_ in range(2):
                for s_ in range(2):
                    nc.vector.reciprocal(
                        rc[tt_][D * s_ : D * (s_ + 1), :, :],
                        at_ps[tt_][64 * s_ + D : 64 * s_ + 2 * D, :, :],
                    )

            attnT_sb = atsb.tile([P, 4, P], F32, tag="attnT")
            for h in range(H):
                tt_ = h // 8
                s_ = h % 2
                j_ = (h % 8) // 2
                g, ro = h // 4, D * (h % 4)
                nc.vector.tensor_tensor(
                    attnT_sb[ro : ro + D, g, :],
                    at_ps[tt_][64 * s_ : 64 * s_ + D, j_, :],
                    rc[tt_][D * s_ : D * (s_ + 1), j_, :],
                    op=MULT,
                )

            # ---- output projection
            ps_o = ps_mi.tile([P, N], F32, tag="mi")
            for g in range(4):
                nc.tensor.matmul(
                    ps_o[:],
                    attnT_sb[:, g, :],
                    w_sb["woT"][g][:],
                    start=(g == 0),
                    stop=False,
                )
            nc.tensor.matmul(ps_o[:], ones_r[:], bo_row[:], start=False, stop=True)
            o_sb = opool.tile([P, N], F32, tag="o")
            nc.scalar.copy(o_sb[:], ps_o[:])
            nc.sync.dma_start(t["out"][b, P * qt : P * (qt + 1), :], o_sb[:])


_PROG = None


def _get_prog(reps=1):
    global _PROG
    if reps != 1:
        return _build_prog(reps)
    if _PROG is None:
        _PROG = _build_prog(1)
    return _PROG


def _build_prog(reps):
        nc = bacc.Bacc("TRN2", target_bir_lowering=False, debug=False,
                       num_devices=NCORES)
        t = {
            "nfeat": nc.dram_tensor("nfeat", [BLOC, N, F], F32, kind="ExternalInput").ap(),
            "attn_bias": nc.dram_tensor("attn_bias", [BLOC, N, N, H], F32, kind="ExternalInput").ap(),
            "attn_mask": nc.dram_tensor("attn_mask", [BLOC, N, N], I32, kind="ExternalInput").ap(),
            "wqT": nc.dram_tensor("wqT", [F, F], F32, kind="ExternalInput").ap(),
            "wkT": nc.dram_tensor("wkT", [F, F], F32, kind="ExternalInput").ap(),
            "wvT": nc.dram_tensor("wvT", [F, F], F32, kind="ExternalInput").ap(),
            "woT": nc.dram_tensor("woT", [F, F], F32, kind="ExternalInput").ap(),
            "bqs": nc.dram_tensor("bqs", [F], F32, kind="ExternalInput").ap(),
            "bk": nc.dram_tensor("bk", [F], F32, kind="ExternalInput").ap(),
            "bv": nc.dram_tensor("bv", [F], F32, kind="ExternalInput").ap(),
            "bo": nc.dram_tensor("bo", [F], F32, kind="ExternalInput").ap(),
            "out": nc.dram_tensor("out", [BLOC, N, F], F32, kind="ExternalOutput").ap(),
        }
        with tile.TileContext(nc) as tc, ExitStack() as ctx:
            _emit(nc, tc, ctx, t, reps=reps)
        nc.compile()
        return nc


def kernel(nfeat, attn_bias, attn_mask, Wq, bq, Wk, bk, Wv, bv, Wo, bo):
    nc = _get_prog()
    nfeat = np.ascontiguousarray(np.asarray(nfeat, dtype=np.float32))
    attn_bias = np.ascontiguousarray(np.asarray(attn_bias, dtype=np.float32))
    attn_mask = np.ascontiguousarray(np.asarray(attn_mask, dtype=np.int32))
    shared = {
        "wqT": np.ascontiguousarray(np.asarray(Wq, dtype=np.float32).T),
        "wkT": np.ascontiguousarray(np.asarray(Wk, dtype=np.float32).T),
        "wvT": np.ascontiguousarray(np.asarray(Wv, dtype=np.float32).T),
        "woT": np.ascontiguousarray(np.asarray(Wo, dtype=np.float32).T),
        "bqs": np.asarray(bq, dtype=np.float32) * SQRT_D,
        "bk": np.asarray(bk, dtype=np.float32),
        "bv": np.asarray(bv, dtype=np.float32),
        "bo": np.asarray(bo, dtype=np.float32),
    }
    in_maps = []
    for c in range(NCORES):
        m = dict(shared)
        m["nfeat"] = nfeat[BLOC * c : BLOC * (c + 1)]
        m["attn_bias"] = attn_bias[BLOC * c : BLOC * (c + 1)]
        m["attn_mask"] = attn_mask[BLOC * c : BLOC * (c + 1)]
        in_maps.append(m)

    kernel.last_in_maps = in_maps
    trace = bool(int(os.environ.get("KERNEL_TRACE", "0")))
    res = run_bass_kernel_spmd(
        nc, in_maps, core_ids=list(range(NCORES)), trace=trace
    )
    if trace:
        kernel.last_exec_time_ns = res.exec_time_ns
        kernel.last_profile = res.profile_json
    out = np.concatenate([r["out"] for r in res.results], axis=0)
    return out.astype(np.float32)


kernel.last_exec_time_ns = None
kernel.last_profile = None
kernel.last_in_maps = None


# revision 13
# speedup vs baseline: 4.2442x; 1.1847x over previous
"""BiasedMHA Trainium2 kernel.

Full inputs -> shard batch over 8 NeuronCores -> Bass/Tile kernel -> gather.

Reference semantics (B=16, N=512, F=512, H=16, D=32):
  q = (x @ Wq.T + bq) * sqrt(D); k = x @ Wk.T + bk; v = x @ Wv.T + bv
  s[b,q,k,h] = sum_d q.k + bias[b,q,k,h];  s = -inf where mask[b,q,k]!=0
  p = softmax_k(s);  out = (p @ v reshaped) @ Wo.T + bo

Per-core design notes:
 - X^T via PE transpose; projections as W^T-stationary fp32 matmuls.
 - V kept in natural (n, f) layout, augmented with a ones column per head so
   the P@V matmul also emits the softmax denominator (M=33) for free.
 - Scores stay q-major for the (q,k,h)-contiguous bias tile add + int mask
   predication, then are PE-transposed to k-major so the exp (ScalarE) writes
   P^T straight to SBUF for the P@V matmul - no PSUM->SBUF copy for P.
 - softmax uses a fixed exp shift (exp(s - C)) instead of a row max: scores
   are bounded (std ~16) so exp stays in fp32 range and the shift cancels.
 - Normalization (1/rowsum) is folded into the attn^T PSUM->SBUF copies.
"""

import os
import numpy as np
from contextlib import ExitStack

import concourse.bass as bass
import concourse.mybir as mybir
import concourse.tile as tile
from concourse import bacc
from concourse.bass_utils import run_bass_kernel_spmd
from concourse.masks import make_identity

F32 = mybir.dt.float32
F32R = mybir.dt.float32r
I32 = mybir.dt.int32
ADD = mybir.AluOpType.add
MULT = mybir.AluOpType.mult
AF = mybir.ActivationFunctionType

B, N, F, H = 16, 512, 512, 16
D = F // H            # 32
NCORES = 8
BLOC = B // NCORES    # 2
P = 128
QT = N // P           # 4 q tiles
KC = N // P           # 4 k chunks
SQRT_D = float(np.sqrt(D))
C_EXP = 90.0          # fixed softmax shift; |scores| << C_EXP + 87 (fp32 safe)
NEG_HUGE = -1.0e30


def _emit(nc, tc, ctx, t, reps=1):
    consts = ctx.enter_context(tc.tile_pool(name="consts", bufs=1))
    wpool = ctx.enter_context(tc.tile_pool(name="weights", bufs=1))
    xpool = ctx.enter_context(tc.tile_pool(name="x", bufs=5))
    bpool = ctx.enter_context(tc.tile_pool(name="perbatch", bufs=1))
    biaspool = ctx.enter_context(tc.tile_pool(name="bias", bufs=2))
    maskpool = ctx.enter_context(tc.tile_pool(name="mask", bufs=2))
    mcpool = ctx.enter_context(tc.tile_pool(name="maskC", bufs=2))
    spool = ctx.enter_context(tc.tile_pool(name="sprime", bufs=4))
    ppool = ctx.enter_context(tc.tile_pool(name="pT", bufs=4))
    atsb = ctx.enter_context(tc.tile_pool(name="attnT", bufs=2))
    opool = ctx.enter_context(tc.tile_pool(name="o", bufs=2))
    rspool = ctx.enter_context(tc.tile_pool(name="rs", bufs=2))

    ps_sc = ctx.enter_context(tc.tile_pool(name="ps_sc", bufs=3, space="PSUM"))
    ps_at = ctx.enter_context(tc.tile_pool(name="ps_at", bufs=3, space="PSUM"))
    ps_mi = ctx.enter_context(tc.tile_pool(name="ps_mi", bufs=2, space="PSUM"))

    ident = consts.tile([P, P], F32)
    make_identity(nc, ident[:])
    neghuge = consts.tile([P, 1], F32)
    nc.vector.memset(neghuge[:], NEG_HUGE)
    ones_col = consts.tile([1, P], F32)
    nc.vector.memset(ones_col[:], 1.0)
    ones_r = consts.tile([1, P], F32R)
    nc.vector.tensor_copy(ones_r[:], ones_col[:])
    negc = consts.tile([P, 1], F32)
    nc.vector.memset(negc[:], -C_EXP)

    # per-partition bias vectors for Q/K projection epilogues
    bqs_sb = consts.tile([P, 4], F32)
    nc.sync.dma_start(bqs_sb[:], t["bqs"].rearrange("(a p) -> p a", p=P))
    bk_sb = consts.tile([P, 4], F32)
    nc.sync.dma_start(bk_sb[:], t["bk"].rearrange("(a p) -> p a", p=P))
    bv_row0 = consts.tile([1, F], F32)
    nc.sync.dma_start(bv_row0[:], t["bv"].rearrange("(a f) -> a f", a=1))
    bv_row = consts.tile([1, F], F32R)
    nc.vector.tensor_copy(bv_row[:], bv_row0[:])
    bo_row0 = consts.tile([1, F], F32)
    nc.sync.dma_start(bo_row0[:], t["bo"].rearrange("(a f) -> a f", a=1))
    bo_row = consts.tile([1, F], F32R)
    nc.vector.tensor_copy(bo_row[:], bo_row0[:])

    # prefetch the first batch's X tiles ahead of the (bulky) weight DMAs so
    # the PE transposes can start immediately
    x_prefetch = []
    for nb in range(4):
        xt_ = xpool.tile([P, F], F32, tag="x", name=f"xpre{nb}")
        nc.sync.dma_start(xt_[:], t["nfeat"][0, P * nb : P * (nb + 1), :])
        x_prefetch.append(xt_)

    w_sb = {}
    for name in ("wqT", "wkT", "wvT", "woT"):
        w_sb[name] = []
        for ki in range(4):
            wt = wpool.tile([P, F], F32, tag=f"{name}{ki}")
            nc.sync.dma_start(wt[:], t[name][P * ki : P * (ki + 1), :])
            w_sb[name].append(wt)

    for rep in range(reps):
      for b in range(BLOC):
        # ---- X load + transpose to (f_in, n)
        if rep == 0 and b == 0:
            x_tiles = x_prefetch
        else:
            x_tiles = []
            for nb in range(4):
                xt_ = xpool.tile([P, F], F32, tag="x")
                nc.sync.dma_start(xt_[:], t["nfeat"][b, P * nb : P * (nb + 1), :])
                x_tiles.append(xt_)
        xT_sb = bpool.tile([P, 4, N], F32, tag="xT")
        for fb in range(4):
            ps = ps_mi.tile([P, N], F32, tag="mi")
            for nb in range(4):
                nc.tensor.transpose(
                    ps[:, P * nb : P * (nb + 1)],
                    x_tiles[nb][:, P * fb : P * (fb + 1)],
                    ident[:],
                )
            nc.scalar.copy(xT_sb[:, fb, :], ps[:])

        # ---- Q/K projections -> (f_out, n), V -> natural (n, f) augmented
        qT_sb = bpool.tile([P, 4, N], F32, tag="qT")
        kT_sb = bpool.tile([P, 4, N], F32, tag="kT")
        for wname, dest, scale, bvec in (
            ("wqT", qT_sb, SQRT_D, bqs_sb),
            ("wkT", kT_sb, 1.0, bk_sb),
        ):
            for fo in range(4):
                ps = ps_mi.tile([P, N], F32, tag="mi")
                for ki in range(4):
                    nc.tensor.matmul(
                        ps[:],
                        w_sb[wname][ki][:, P * fo : P * (fo + 1)],
                        xT_sb[:, ki, :],
                        start=(ki == 0),
                        stop=(ki == 3),
                    )
                nc.scalar.activation(
                    dest[:, fo, :], ps[:], AF.Identity,
                    bias=bvec[:, fo : fo + 1], scale=scale,
                )

        v_aug = bpool.tile([P, 4, H, 2 * D], F32, tag="vaug")
        for nb in range(4):
            ps = ps_mi.tile([P, N], F32, tag="mi")
            for ki in range(4):
                nc.tensor.matmul(
                    ps[:],
                    xT_sb[:, ki, P * nb : P * (nb + 1)],
                    w_sb["wvT"][ki][:],
                    start=(ki == 0),
                    stop=False,
                )
            nc.tensor.matmul(ps[:], ones_r[:], bv_row[:], start=False, stop=True)
            nc.scalar.copy(
                v_aug[:, nb, :, 0:D], ps[:].rearrange("p (h d) -> p h d", h=H)
            )
            nc.vector.memset(v_aug[:, nb, :, D : 2 * D], 1.0)

        # ---- attention per q-tile
        for qt in range(QT):
            bias_t = biaspool.tile([P, N, H], F32, tag="bias")
            nc.sync.dma_start(bias_t[:], t["attn_bias"][b, P * qt : P * (qt + 1), :, :])
            mask_t = maskpool.tile([P, N], I32, tag="mask")
            nc.sync.dma_start(mask_t[:], t["attn_mask"][b, P * qt : P * (qt + 1), :])
            maskf = mcpool.tile([P, N], F32, tag="maskf")
            nc.gpsimd.tensor_copy(maskf[:], mask_t[:])
            maskC = mcpool.tile([P, N, 1], F32, tag="maskC")
            nc.vector.tensor_scalar(
                maskC[:, :, 0], maskf[:], 0.0, NEG_HUGE,
                op0=mybir.AluOpType.not_equal, op1=MULT,
            )
            nc.gpsimd.tensor_tensor(
                bias_t[:], bias_t[:], maskC[:].to_broadcast([P, N, H]), op=ADD
            )

            at_ps = [ps_at.tile([P, 4, P], F32, tag="at", name=f"at{qt}_{i}") for i in range(2)]

            for h in range(H):
                tt_ = h // 8
                s_ = h % 2
                j_ = (h % 8) // 2
                fo, ro = h // 4, D * (h % 4)

                sc = ps_sc.tile([P, N], F32, tag="sc")
                kwargs = {}
                if ro == 96:
                    kwargs["tile_position"] = (ro, 0)
                nc.tensor.matmul(
                    sc[:],
                    qT_sb[ro : ro + D, fo, P * qt : P * (qt + 1)],
                    kT_sb[ro : ro + D, fo, :],
                    start=True,
                    stop=True,
                    **kwargs,
                )
                sp = spool.tile([P, N], F32, tag="sp")
                nc.vector.tensor_tensor(sp[:], sc[:], bias_t[:, :, h], op=ADD)

                for c in range(4):
                    nc.tensor.transpose(
                        sc[:, P * c : P * (c + 1)], sp[:, P * c : P * (c + 1)], ident[:]
                    )
                pT = ppool.tile([P, 4, P], F32, tag="pT")
                nc.scalar.activation(
                    pT[:], sc[:].rearrange("p (c q) -> p c q", c=4),
                    AF.Exp, bias=negc[:], scale=1.0,
                )

                for kc in range(4):
                    nc.tensor.matmul(
                        at_ps[tt_][64 * s_ : 64 * s_ + 2 * D, j_, :],
                        v_aug[:, kc, h, :],
                        pT[:, kc, :],
                        start=(kc == 0),
                        stop=(kc == 3),
                        tile_position=(0, 64 * s_),
                    )

            # ---- replicated rowsums -> reciprocal, then normalized attn^T
            rc = [rspool.tile([2 * D, 4, P], F32, tag=f"rc{i}", name=f"rc{qt}_{i}") for i in range(2)]
            for tt_ in range(2):
                for s_ in range(2):
                    nc.vector.reciprocal(
                        rc[tt_][D * s_ : D * (s_ + 1), :, :],
                        at_ps[tt_][64 * s_ + D : 64 * s_ + 2 * D, :, :],
                    )

            attnT_g = [atsb.tile([P, P], F32, tag=f"attnT{g}", name=f"attnT{qt}_{g}")
                       for g in range(4)]
            for h in range(H):
                tt_ = h // 8
                s_ = h % 2
                j_ = (h % 8) // 2
                g, ro = h // 4, D * (h % 4)
                nc.vector.tensor_tensor(
                    attnT_g[g][ro : ro + D, :],
                    at_ps[tt_][64 * s_ : 64 * s_ + D, j_, :],
                    rc[tt_][D * s_ : D * (s_ + 1), j_, :],
                    op=MULT,
                )

            # ---- output projection
            ps_o = ps_mi.tile([P, N], F32, tag="mi")
            for g in range(4):
                nc.tensor.matmul(
                    ps_o[:],
                    attnT_g[g][:],
                    w_sb["woT"][g][:],
                    start=(g == 0),
                    stop=False,
                )
            nc.tensor.matmul(ps_o[:], ones_r[:], bo_row[:], start=False, stop=True)
            o_sb = opool.tile([P, N], F32, tag="o")
            nc.scalar.copy(o_sb[:], ps_o[:])
            nc.sync.dma_start(t["out"][b, P * qt : P * (qt + 1), :], o_sb[:])


_PROG = None


def _get_prog(reps=1):
    global _PROG
    if reps != 1:
        return _build_prog(reps)
    if _PROG is None:
        _PROG = _build_prog(1)
    return _PROG


def _build_prog(reps):
        nc = bacc.Bacc("TRN2", target_bir_lowering=False, debug=False,
                       num_devices=NCORES)
        t = {
            "nfeat": nc.dram_tensor("nfeat", [BLOC, N, F], F32, kind="ExternalInput").ap(),
            "attn_bias": nc.dram_tensor("attn_bias", [BLOC, N, N, H], F32, kind="ExternalInput").ap(),
            "attn_mask": nc.dram_tensor("attn_mask", [BLOC, N, N], I32, kind="ExternalInput").ap(),
            "wqT": nc.dram_tensor("wqT", [F, F], F32, kind="ExternalInput").ap(),
            "wkT": nc.dram_tensor("wkT", [F, F], F32, kind="ExternalInput").ap(),
            "wvT": nc.dram_tensor("wvT", [F, F], F32, kind="ExternalInput").ap(),
            "woT": nc.dram_tensor("woT", [F, F], F32, kind="ExternalInput").ap(),
            "bqs": nc.dram_tensor("bqs", [F], F32, kind="ExternalInput").ap(),
            "bk": nc.dram_tensor("bk", [F], F32, kind="ExternalInput").ap(),
            "bv": nc.dram_tensor("bv", [F], F32, kind="ExternalInput").ap(),
            "bo": nc.dram_tensor("bo", [F], F32, kind="ExternalInput").ap(),
            "out": nc.dram_tensor("out", [BLOC, N, F], F32, kind="ExternalOutput").ap(),
        }
        with tile.TileContext(nc) as tc, ExitStack() as ctx:
            _emit(nc, tc, ctx, t, reps=reps)
        nc.compile()
        return nc


def kernel(nfeat, attn_bias, attn_mask, Wq, bq, Wk, bk, Wv, bv, Wo, bo):
    nc = _get_prog()
    nfeat = np.ascontiguousarray(np.asarray(nfeat, dtype=np.float32))
    attn_bias = np.ascontiguousarray(np.asarray(attn_bias, dtype=np.float32))
    attn_mask = np.ascontiguousarray(np.asarray(attn_mask, dtype=np.int32))
    shared = {
        "wqT": np.ascontiguousarray(np.asarray(Wq, dtype=np.float32).T),
        "wkT": np.ascontiguousarray(np.asarray(Wk, dtype=np.float32).T),
        "wvT": np.ascontiguousarray(np.asarray(Wv, dtype=np.float32).T),
        "woT": np.ascontiguousarray(np.asarray(Wo, dtype=np.float32).T),
        "bqs": np.asarray(bq, dtype=np.float32) * SQRT_D,
        "bk": np.asarray(bk, dtype=np.float32),
        "bv": np.asarray(bv, dtype=np.float32),
        "bo": np.asarray(bo, dtype=np.float32),
    }
    in_maps = []
    for c in range(NCORES):
        m = dict(shared)
        m["nfeat"] = nfeat[BLOC * c : BLOC * (c + 1)]
        m["attn_bias"] = attn_bias[BLOC * c : BLOC * (c + 1)]
        m["attn_mask"] = attn_mask[BLOC * c : BLOC * (c + 1)]
        in_maps.append(m)

    kernel.last_in_maps = in_maps
    trace = bool(int(os.environ.get("KERNEL_TRACE", "0")))
    res = run_bass_kernel_spmd(
        nc, in_maps, core_ids=list(range(NCORES)), trace=trace
    )
    if trace:
        kernel.last_exec_time_ns = res.exec_time_ns
        kernel.last_profile = res.profile_json
    out = np.concatenate([r["out"] for r in res.results], axis=0)
    return out.astype(np.float32)


kernel.last_exec_time_ns = None
kernel.last_profile = None
kernel.last_in_maps = None
